# revision 2
# baseline (speedup 1.0000x reference)
"""Causal GRN-EMA normalization kernel for 8x TRN2 NeuronCores (Bass/Tile).

Math (per batch b, channel c, time t):
    ema_t   = ALPHA*ema_{t-1} + (1-ALPHA)*x_t^2,  ema_{-1} = EMA_INIT
    ema_hat = ema_t / (1 - ALPHA^{t+1} + EPS)
    g       = sqrt(ema_hat + EPS)
    n       = g / (mean_c(g) + EPS)
    y       = gamma*(x*n) + beta + x

Strategy: data-parallel over B (16 batches -> 2 per core). The T-recurrence
is a blocked scan on the tensor engine with carry depth D (one PSUM->SBUF
carry copy per D=4 blocks):
    block q in group: ema_q = sum_{d=0..q} W_d @ bsq_{q-d} + A^{qL+i+1} (x) E
with W_0 the lower-tri within-block weights and W_d (d>=1) dense rank-1
cross-block weights.  x/y are shipped bf16 (halves HBM traffic; the rel-err
budget of 2e-2 tolerates ~5e-3).  x is pre-rotated on the host (partition p
holds time (p-1) mod 128) so the group-carry row lands on partition 0, and
the output is un-rotated (+beta) on the host.
"""

import os
from contextlib import ExitStack

import numpy as np

ALPHA = 0.99
EPS = 1e-6
EMA_INIT = 1e-4

B, T, C = 16, 8192, 512
NCORES = 8
BPC = B // NCORES          # batches per core
L = 128                    # scan block (partition dim)
NBLK = T // L              # 64 blocks per batch

DEFAULT_CFG = dict(
    chunk=4,           # blocks per DMA chunk
    depth=4,           # carry depth: blocks per E-copy group (divides chunk)
    interleave=True,   # interleave the two batches' chunk streams
    ecopy="pool",      # engine for the group E-copy: "pool" | "act" | "dve"
    accum_pool_every=2,  # every Nth block's channel-sum runs on Pool (0=off)
    xin_bufs=6,
    bsq_bufs=3,
    g_bufs=4,
    ab_bufs=3,
    y_bufs=6,
    e_bufs=4,
    stat_bufs=8,
    pblk_bufs=4,
    x_observer=True,
    gt_observer=True,
    prefetch_head=2,   # DMA the first N chunks' x before the constants
    y_split=1,         # split the per-chunk y-out DMA into N pieces
    ablate_dma=False,
    ablate_compute=False,
)

_cache = {}


def _host_constants(depth):
    # Partition rotation: partition p holds time index rot[p] = (p-1) mod L,
    # so the block/group-carry row (time L-1) lands on partition 0.
    i = np.arange(L, dtype=np.float64)
    jj, ii = np.meshgrid(i, i, indexing="ij")
    rot = (np.arange(L) - 1) % L
    # W_0[j, i] = (1-A) * A^(i-j) for j <= i else 0 (within-block weights);
    # W_d[j, i] = (1-A) * A^(d*L + i - j) dense (cross-block weights).
    # Both matmul operands live in rotated partition order, so permute both
    # axes; stack the depth matrices side by side -> [L, depth*L].
    ws = []
    w0 = np.where(jj <= ii, (1.0 - ALPHA) * ALPHA ** (ii - jj), 0.0)
    ws.append(w0[np.ix_(rot, rot)])
    for d in range(1, depth):
        wd = (1.0 - ALPHA) * ALPHA ** (d * L + ii - jj)
        ws.append(wd[np.ix_(rot, rot)])
    wmat = np.concatenate(ws, axis=1)  # [L, depth*L]
    # powq[0, q*L + p] = A^(q*L + rot[p] + 1): E-carry decay for block q of
    # the group.
    powq = np.concatenate(
        [ALPHA ** (q * L + i[rot] + 1.0) for q in range(depth)]
    )[None, :]  # [1, depth*L]
    # rden[p, k] = 1 / (1 - A^(128k + rot[p] + 1) + EPS)
    k = np.arange(NBLK, dtype=np.float64)
    tg = 128.0 * k[None, :] + i[rot][:, None] + 1.0
    rden = 1.0 / (1.0 - ALPHA**tg + EPS)
    return (
        np.ascontiguousarray(wmat),
        np.ascontiguousarray(powq),
        np.ascontiguousarray(rden.astype(np.float32)),
    )


def _build_nc(repeat=1, cfg=None):
    import concourse.bacc as bacc
    import concourse.bass as bass
    import concourse.mybir as mybir
    import concourse.tile as tile

    cfg = {**DEFAULT_CFG, **(cfg or {})}
    CHUNK = cfg["chunk"]
    D = cfg["depth"]
    NCHUNK = NBLK // CHUNK
    assert NCHUNK * CHUNK == NBLK
    assert CHUNK % D == 0
    NGRP = CHUNK // D          # carry groups per chunk

    f32 = mybir.dt.float32
    bf16 = mybir.dt.bfloat16

    nc = bacc.Bacc()
    x_h = nc.dram_tensor("x", [BPC, T, C], bf16, kind="ExternalInput")
    gamma_h = nc.dram_tensor("gamma", [1, C], bf16, kind="ExternalInput")
    wmat_h = nc.dram_tensor("wmat", [L, D * L], bf16, kind="ExternalInput")
    powq_h = nc.dram_tensor("powq", [1, D * L], bf16, kind="ExternalInput")
    rden_h = nc.dram_tensor("rden", [L, NBLK], f32, kind="ExternalInput")
    einit_h = nc.dram_tensor("einit", [1, C], bf16, kind="ExternalInput")
    y_h = nc.dram_tensor("y", [BPC, T, C], bf16, kind="ExternalOutput")

    with tile.TileContext(nc) as tc, ExitStack() as ctx:
        singles = ctx.enter_context(tc.tile_pool(name="singles", bufs=1))
        xin = ctx.enter_context(tc.tile_pool(name="xin", bufs=cfg["xin_bufs"]))
        bsqp = ctx.enter_context(tc.tile_pool(name="bsqp", bufs=cfg["bsq_bufs"]))
        gp = ctx.enter_context(tc.tile_pool(name="gp", bufs=cfg["g_bufs"]))
        abp = ctx.enter_context(tc.tile_pool(name="abp", bufs=cfg["ab_bufs"]))
        yp = ctx.enter_context(tc.tile_pool(name="yp", bufs=cfg["y_bufs"]))
        ep = ctx.enter_context(tc.tile_pool(name="ep", bufs=cfg["e_bufs"]))
        statp = ctx.enter_context(tc.tile_pool(name="statp", bufs=cfg["stat_bufs"]))

        # --- head prefetch: start the first x transfers before anything ---
        prefetched = {}
        if cfg["prefetch_head"]:
            order = []
            if cfg["interleave"] and BPC == 2:
                for ci in range(NCHUNK):
                    order += [(0, ci), (1, ci)]
            else:
                order = [(b, ci) for b in range(BPC) for ci in range(NCHUNK)]
            for b0, c0 in order[: cfg["prefetch_head"]]:
                px = xin.tile([L, CHUNK, C], bf16, name=f"pf{b0}_{c0}", tag="xt")
                nc.sync.dma_start(
                    out=px,
                    in_=x_h[b0, c0 * CHUNK * L : (c0 + 1) * CHUNK * L, :].rearrange(
                        "(n p) c -> p n c", p=L
                    ),
                )
                prefetched[(b0, c0)] = px

        # --- constants, loaded once ---
        wmat_s = singles.tile([L, D * L], bf16)
        nc.sync.dma_start(out=wmat_s, in_=wmat_h[:, :])
        powq_s = singles.tile([1, D * L], bf16)
        nc.sync.dma_start(out=powq_s, in_=powq_h[:, :])
        rden_s = singles.tile([L, NBLK], f32)
        nc.sync.dma_start(out=rden_s, in_=rden_h[:, :])
        # kernel() ships gamma*C so rm = 1/s works without the extra
        # (s/C + EPS) tensor_scalar.
        gamma_s = singles.tile([L, C], bf16)
        nc.sync.dma_start(
            out=gamma_s,
            in_=bass.AP(tensor=gamma_h, offset=0, ap=[[0, L], [1, C]]),
        )
        e_init = singles.tile([1, C], bf16)
        nc.sync.dma_start(out=e_init, in_=einit_h[:, :])
        eps_s = singles.tile([L, 1], f32)
        nc.vector.memset(eps_s, EPS)

        # Engine warm-ups: absorb the constant-DMA/memset waits into each
        # engine's vector clock (HW sync-wait slots per instruction are
        # extremely limited; Bacc legalizes overflow with event-semaphore
        # chains, but those cost latency in the steady state).
        wpsum = ctx.enter_context(tc.tile_pool(name="wpsum", bufs=1, space="PSUM"))
        warm = [
            (wmat_s[:, 0:L], wmat_s[:, 0:1]),
            (powq_s[:, 0:1], powq_s[:, 0:L]),
            (e_init[:, 0:L], e_init[:, 0:1]),
        ]
        for wi, (wl, wr) in enumerate(warm):
            wup = wpsum.tile([L, L], f32, tag="warmup", name=f"wup{wi}")
            nc.tensor.matmul(
                wup[: wl.shape[-1], : wr.shape[-1]],
                wl, wr,
                start=True, stop=True,
            )
        psum = ctx.enter_context(
            tc.tile_pool(name="psum", bufs=cfg["pblk_bufs"], space="PSUM")
        )
        scr_act = singles.tile([L, 1], f32)
        nc.scalar.copy(out=scr_act, in_=rden_s[:, 0:1])
        scr_act2 = singles.tile([L, 1], f32)
        nc.scalar.copy(out=scr_act2, in_=eps_s)
        scr_dve = singles.tile([L, 1], bf16)
        nc.vector.tensor_copy(out=scr_dve, in_=gamma_s[:, 0:1])
        scr_pool = singles.tile([L, 1], bf16)
        nc.gpsimd.tensor_copy(out=scr_pool, in_=gamma_s[:, 1:2])
        obsp = ctx.enter_context(tc.tile_pool(name="obsp", bufs=2))

        # chunk schedule
        sched = []
        for _ in range(repeat):
            if cfg["interleave"] and BPC == 2:
                for ci in range(NCHUNK):
                    sched.append((0, ci))
                    sched.append((1, ci))
            else:
                for b in range(BPC):
                    for ci in range(NCHUNK):
                        sched.append((b, ci))

        e_cur = {}
        blk_idx = 0
        for b, ci in sched:
            if ci == 0:
                e_cur[b] = e_init
            t0 = ci * CHUNK * L
            x_view = x_h[b, t0 : t0 + CHUNK * L, :].rearrange(
                "(n p) c -> p n c", p=L
            )
            y_view = y_h[b, t0 : t0 + CHUNK * L, :].rearrange(
                "(n p) c -> p n c", p=L
            )

            if (b, ci) in prefetched:
                xt = prefetched.pop((b, ci))
            else:
                xt = xin.tile([L, CHUNK, C], bf16)
                if cfg["ablate_dma"]:
                    nc.sync.dma_start(
                        out=xt[0:1, 0, 0:1], in_=x_view[0:1, 0, 0:1]
                    )
                else:
                    nc.sync.dma_start(out=xt, in_=x_view)
            if cfg["x_observer"]:
                # DVE observer: cover the x-DMA semaphore on DVE's clock so
                # the per-chunk TSP that reads xt keeps <=2 waits.
                obs = obsp.tile([1, 1], bf16)
                nc.vector.tensor_copy(out=obs, in_=xt[0:1, 0, 0:1])

            # x^2 for the whole chunk in one DVE TSP (4x bf16 mode)
            bsq = bsqp.tile([L, CHUNK, C], bf16)
            if cfg["ablate_compute"]:
                nc.vector.scalar_tensor_tensor(
                    out=bsq[0:1, 0, 0:1], in0=xt[0:1, 0, 0:1], scalar=1.0,
                    in1=xt[0:1, 0, 0:1],
                    op0=mybir.AluOpType.mult, op1=mybir.AluOpType.mult,
                )
            else:
                nc.vector.scalar_tensor_tensor(
                    out=bsq, in0=xt, scalar=1.0, in1=xt,
                    op0=mybir.AluOpType.mult, op1=mybir.AluOpType.mult,
                )

            gt = gp.tile([L, CHUNK, C], bf16)
            yt = yp.tile([L, CHUNK, C], bf16)
            # Pool observer: a dummy write into the fresh yt slot absorbs
            # the y-out DMA's slot-release semaphore on Pool's clock.
            nc.gpsimd.memset(yt[0:1, 0, 0:1], 0.0)
            if cfg["gt_observer"]:
                # ACT observer: dummy write into the fresh gt slot absorbs the
                # DVE slot-release wait, keeping the AP-bias Sqrt at 1 wait.
                nc.scalar.copy(out=gt[0:1, 0, 0:1], in_=eps_s[0:1, :])

            for gi in range(NGRP):
                ptjs = []
                # group matmuls: block q needs q+1 W-matmuls + the E-decay
                for q in range(D):
                    j = gi * D + q
                    kblk = ci * CHUNK + j
                    ptj = psum.tile([L, C], f32, tag="pblk", name=f"pb{blk_idx + q}")
                    ptjs.append(ptj)
                    nc.tensor.matmul(
                        ptj, powq_s[:, q * L : (q + 1) * L], e_cur[b][:, :],
                        start=True, stop=False,
                    )
                    for d in range(q + 1):
                        nc.tensor.matmul(
                            ptj,
                            wmat_s[:, d * L : (d + 1) * L],
                            bsq[:, gi * D + (q - d), :],
                            start=False, stop=(d == q),
                        )
                # group carry out: last row of block D-1's ema (partition 0,
                # rotated layout)
                e_next = ep.tile([1, C], bf16)
                ec = cfg["ecopy"]
                if ec == "pool":
                    nc.gpsimd.tensor_copy(out=e_next, in_=ptjs[D - 1][0:1, :])
                elif ec == "act":
                    nc.scalar.copy(out=e_next, in_=ptjs[D - 1][0:1, :])
                else:
                    nc.vector.tensor_copy(out=e_next, in_=ptjs[D - 1][0:1, :])

                for q in range(D):
                    j = gi * D + q
                    kblk = ci * CHUNK + j
                    ptj = ptjs[q]
                    if cfg["ablate_compute"]:
                        nc.scalar.copy(out=gt[0:1, j, 0:1], in_=ptj[0:1, 0:1])
                        nc.vector.scalar_tensor_tensor(
                            out=yt[0:1, j, 0:1], in0=gt[0:1, j, 0:1], scalar=1.0,
                            in1=xt[0:1, j, 0:1],
                            op0=mybir.AluOpType.add, op1=mybir.AluOpType.mult,
                        )
                        blk_idx += 1
                        continue
                    # g = sqrt(ema * rden + EPS), s = sum_c g
                    s = statp.tile([L, 1], f32)
                    ape = cfg["accum_pool_every"]
                    if ape and (blk_idx % ape == 0):
                        nc.scalar.activation(
                            out=gt[:, j, :],
                            in_=ptj,
                            func=mybir.ActivationFunctionType.Sqrt,
                            bias=eps_s,
                            scale=rden_s[:, kblk : kblk + 1],
                        )
                        mscr = abp.tile([L, C], bf16, tag="mscr")
                        nc.gpsimd.tensor_scalar(
                            out=mscr, in0=gt[:, j, :], scalar1=1.0, scalar2=None,
                            op0=mybir.AluOpType.mult, accum_out=s,
                        )
                    else:
                        nc.scalar.activation(
                            out=gt[:, j, :],
                            in_=ptj,
                            func=mybir.ActivationFunctionType.Sqrt,
                            bias=eps_s,
                            scale=rden_s[:, kblk : kblk + 1],
                            accum_out=s,
                        )
                    # rm = 1/s; the /C is folded into gamma on the host
                    rm = statp.tile([L, 1], f32)
                    nc.vector.reciprocal(out=rm, in_=s)
                    # at = (g * rm) * gamma
                    at = abp.tile([L, C], bf16)
                    nc.vector.scalar_tensor_tensor(
                        out=at, in0=gt[:, j, :], scalar=rm, in1=gamma_s,
                        op0=mybir.AluOpType.mult, op1=mybir.AluOpType.mult,
                    )
                    # y_dev = (at + 1) * x; +beta happens on the host
                    nc.vector.scalar_tensor_tensor(
                        out=yt[:, j, :], in0=at, scalar=1.0, in1=xt[:, j, :],
                        op0=mybir.AluOpType.add, op1=mybir.AluOpType.mult,
                    )
                    blk_idx += 1
                e_cur[b] = e_next

            # y stays rotated; host un-rotates
            if cfg["ablate_dma"]:
                nc.sync.dma_start(out=y_view[0:1, 0, 0:1], in_=yt[0:1, 0, 0:1])
            else:
                ys = cfg["y_split"]
                step = CHUNK // ys
                for p0 in range(0, CHUNK, step):
                    nc.sync.dma_start(
                        out=y_view[:, p0 : p0 + step, :],
                        in_=yt[:, p0 : p0 + step, :],
                    )
    nc.finalize()
    return nc


def _get_nc():
    if "nc" not in _cache:
        _cache["nc"] = _build_nc()
    return _cache["nc"]


def kernel(x, gamma, beta, _want_profile=False):
    import ml_dtypes
    from concourse.bass_utils import run_bass_kernel_spmd

    bf16 = ml_dtypes.bfloat16
    x = np.asarray(x, dtype=np.float32)
    gamma = np.ascontiguousarray(np.asarray(gamma, dtype=np.float32))
    beta = np.ascontiguousarray(np.asarray(beta, dtype=np.float32))
    assert x.shape == (B, T, C), x.shape
    # pre-rotate: within each 128-step block, partition p holds time (p-1)%128
    xb = x.astype(bf16)
    xb = np.roll(xb.reshape(B, NBLK, L, C), 1, axis=2).reshape(B, T, C)

    depth = DEFAULT_CFG["depth"]
    wmat, powq, rden = _host_constants(depth)
    einit = np.full((1, C), EMA_INIT, dtype=bf16)
    nc = _get_nc()

    # device computes rm = 1/sum_c(g); fold the /C into gamma
    gamma_dev = np.ascontiguousarray((gamma * np.float32(C)).astype(bf16))

    in_maps = []
    for core in range(NCORES):
        xs = np.ascontiguousarray(xb[core * BPC : (core + 1) * BPC])
        in_maps.append(
            {
                "x": xs,
                "gamma": gamma_dev,
                "wmat": wmat.astype(bf16),
                "powq": powq.astype(bf16),
                "rden": rden,
                "einit": einit,
            }
        )

    # NOTE: trace=True requires antenv.axon_hooks, absent in this container.
    res = run_bass_kernel_spmd(nc, in_maps, list(range(NCORES)), trace=False)
    y = np.concatenate(
        [res.results[core]["y"].astype(np.float32) for core in range(NCORES)],
        axis=0,
    )
    # un-rotate, +beta (device skipped it)
    y = np.roll(y.reshape(B, NBLK, L, C), -1, axis=2).reshape(B, T, C)
    y = y + beta[None, :, :]
    y = np.ascontiguousarray(y)
    if _want_profile:
        _cache["last_profile"] = res
    return y
